# revision 13
# baseline (speedup 1.0000x reference)
"""CRF loss kernel for Trainium2 (8 NeuronCores, data-parallel over batch).

reference: mean_b( logZ_b - score_b ) for a linear-chain CRF with
B=256, S=512, T=128.

Denominator (logZ, 99.9% of the FLOPs) runs on device in exp space:
    u_0[t, b]   = exp(start[t]) * exp(em[b, 0, t])
    u_s         = (A^T u_{s-1}) o exp(em_s)          A = exp(transitions)
    logZ_b      = log( sum_t u_S[t,b] * exp(end[t]) ) + C
with a shared (per-core) renormalization scalar folded into the emission
tile every K=8 steps (prescaled off the critical path on the scalar
engine from a 3-step-delayed row sum), accumulated into C. Matmuls are
bf16 with fp32 PSUM accumulation; validated to ~2e-5 relative error
against the fp32 reference.

Layout per core: u is [T=128 partitions, batch free]. The per-step
matmul is out[next, b] = sum_prev A[prev, next] * u[prev, b] with A
stationary. The 32 per-core batches are split into G=2 groups of 16 so
the TensorE -> VectorE -> TensorE dependency chain of the two groups
interleaves; the steady-state step period is the chain-latency floor
(~430 ns: matmul drain + DVE PSUM-access + two semaphore hops).

Numerator (score of the tagged path) is a handful of gathers summing to
~0.1% of the FLOPs; it is computed on the host in fp64.
"""

import numpy as np
import ml_dtypes

B, S, T = 256, 512, 128
NCORES = 8
BC = B // NCORES          # 32 batches per core
G = 2                     # pipeline groups per core
BG = BC // G              # 16 batches per group
RENORM_K = 8              # renorm period (steps)
STAT_LAG = 3              # stat measured at s, applied at s + STAT_LAG
CH0 = 16                  # first chunk kept small so the scan starts early
CH = 64                   # emission chunk length (steps per DMA)

_nc_cache = None
LAST_RESULTS = None       # BassKernelResults of the most recent device run


def _chunks():
    """Chunk layout: (start, length) covering [0, S)."""
    out = [(0, CH0), (CH0, CH - CH0)]
    s = CH
    while s < S:
        out.append((s, CH))
        s += CH
    return out


def _build_nc():
    import concourse.bacc as bacc
    import concourse.mybir as mybir
    import concourse.tile as tile

    fp32 = mybir.dt.float32
    bf16 = mybir.dt.bfloat16
    Exp = mybir.ActivationFunctionType.Exp
    Ln = mybir.ActivationFunctionType.Ln
    Copy = mybir.ActivationFunctionType.Copy
    Identity = mybir.ActivationFunctionType.Identity
    mult = mybir.AluOpType.mult

    nc = bacc.Bacc("TRN2", target_bir_lowering=False, debug=False)

    em_t = nc.dram_tensor("em_t", [T, S, BC], bf16, kind="ExternalInput")
    a_exp = nc.dram_tensor("a_exp", [T, T], bf16, kind="ExternalInput")
    se_exp = nc.dram_tensor("se_exp", [T, 2], fp32, kind="ExternalInput")
    denom = nc.dram_tensor("denom", [1, BC], fp32, kind="ExternalOutput")

    chunks = _chunks()

    with tile.TileContext(nc) as tc:
        with (
            tc.tile_pool(name="const", bufs=1) as constp,
            tc.tile_pool(name="emraw", bufs=3) as emraw_p,
            # all exp(em) chunks stay resident (~4 MB of SBUF)
            tc.tile_pool(name="emexp", bufs=len(chunks)) as emexp_p,
            tc.tile_pool(name="emsc", bufs=3) as emsc_p,
            tc.tile_pool(name="ug0", bufs=2) as up0,
            tc.tile_pool(name="ug1", bufs=2) as up1,
            tc.tile_pool(name="vps", bufs=2, space="PSUM") as vp,
            tc.tile_pool(name="side", bufs=2) as sidep,
        ):
            ups = [up0, up1]

            emexp_tiles = []

            def load_chunk(ci):
                s0, ln = chunks[ci]
                raw = emraw_p.tile([T, ln, BC], bf16)
                nc.sync.dma_start(raw[:], em_t[:, s0:s0 + ln, :])
                ex = emexp_p.tile([T, ln, BC], bf16)
                nc.scalar.activation(ex[:], raw[:], Exp)
                emexp_tiles.append(ex)

            def em_slice(s):
                for ci, (s0, ln) in enumerate(chunks):
                    if s0 <= s < s0 + ln:
                        return emexp_tiles[ci][:, s - s0, :]
                raise AssertionError(s)

            load_chunk(0)

            a_tile = constp.tile([T, T], bf16)
            nc.sync.dma_start(a_tile[:], a_exp[:])
            se_tile = constp.tile([T, 2], fp32)
            nc.sync.dma_start(se_tile[:], se_exp[:])
            ones_t = constp.tile([T, 1], bf16)
            nc.gpsimd.memset(ones_t[:], 1.0)
            c_acc = sidep.tile([1, 1], fp32, tag="cacc")
            nc.gpsimd.memset(c_acc[:], 0.0)
            neg32ln2 = constp.tile([1, 1], fp32)
            nc.gpsimd.memset(neg32ln2[:], float(-32 * np.log(2.0)))
            n_renorm = 0

            for ci in range(1, len(chunks)):
                load_chunk(ci)

            # u_0 = exp(em_0) * exp(start)
            u_cur = []
            for g in range(G):
                u0 = ups[g].tile([T, BG], bf16)
                nc.vector.tensor_scalar(
                    u0[:], em_slice(0)[:, g * BG:(g + 1) * BG],
                    se_tile[:, 0:1], None, mult)
                u_cur.append(u0)

            asum = None
            em_scaled = None
            for s in range(1, S):
                stat_step = ((s + STAT_LAG) % RENORM_K == 0
                             and s + STAT_LAG < S)
                renorm_step = (s % RENORM_K == 0)
                for g in range(G):
                    v = vp.tile([T, BG], fp32, tag=f"v{g}")
                    nc.tensor.matmul(v[:], a_tile[:], u_cur[g][:],
                                     start=True, stop=True)
                    if renorm_step:
                        em_ap = em_scaled[:, g * BG:(g + 1) * BG]
                    else:
                        em_ap = em_slice(s)[:, g * BG:(g + 1) * BG]
                    u_new = ups[g].tile([T, BG], bf16)
                    if stat_step and g == 0:
                        asum = sidep.tile([T, 1], fp32, tag="asum")
                        nc.vector.scalar_tensor_tensor(
                            u_new[:], v[:], 1.0, em_ap, mult, mult,
                            accum_out=asum[:])
                    else:
                        nc.vector.tensor_tensor(u_new[:], v[:], em_ap, mult)
                    u_cur[g] = u_new
                if stat_step:
                    # off-critical-path side chain (ACT + GPSIMD only):
                    # lg = ln(2^-32 * asum[0])   (ACT Ln takes |x| <= 2^64;
                    #                             asum reaches ~e^50)
                    # r  = exp(-lg - 32*ln2) ~= 1/asum[0]
                    # em[s+LAG] *= r;  C += lg  (the 32*ln2 per renorm is a
                    # compile-time count folded into the epilogue constant)
                    lg = sidep.tile([1, 1], fp32, tag="lg")
                    nc.scalar.activation(lg[:], asum[0:1, 0:1], Ln,
                                         scale=float(2.0 ** -32))
                    r_row = sidep.tile([1, 1], fp32, tag="rrow")
                    nc.scalar.activation(r_row[:], lg[:], Exp, scale=-1.0,
                                         bias=neg32ln2[0:1, 0:1])
                    r_bc = sidep.tile([T, 1], fp32, tag="rbc")
                    nc.gpsimd.partition_broadcast(r_bc[:], r_row[:])
                    em_scaled = emsc_p.tile([T, BC], bf16)
                    nc.scalar.activation(em_scaled[:], em_slice(s + STAT_LAG),
                                         Copy, scale=r_bc[:, 0:1])
                    c_new = sidep.tile([1, 1], fp32, tag="cacc")
                    nc.scalar.activation(c_new[:], c_acc[:], Identity,
                                         bias=lg[0:1, 0:1])
                    c_acc = c_new
                    n_renorm += 1

            # epilogue: denom = ln(sum_t u_S * exp(end)) + C
            # (sum via a ones-vector matmul; all matmuls self-load weights,
            # so the ones load cannot corrupt any still-pending scan matmul)
            for g in range(G):
                w = ups[g].tile([T, BG], bf16, tag=f"w{g}")
                nc.vector.tensor_scalar(w[:], u_cur[g][:], se_tile[:, 1:2],
                                        None, mult)
                srow = vp.tile([1, BG], fp32, tag=f"sum{g}")
                nc.tensor.matmul(srow[:], ones_t[:], w[:], start=True, stop=True)
                dlog = sidep.tile([1, BG], fp32, tag=f"dlog{g}")
                nc.scalar.activation(dlog[:], srow[:], Ln,
                                     scale=float(2.0 ** -64))
                dfin = sidep.tile([1, BG], fp32, tag=f"dfin{g}")
                nc.vector.tensor_scalar(
                    dfin[:], dlog[:], c_acc[0:1, 0:1],
                    float((64 + 32 * n_renorm) * np.log(2.0)),
                    mybir.AluOpType.add, mybir.AluOpType.add)
                nc.sync.dma_start(denom[0:1, g * BG:(g + 1) * BG], dfin[:])

    nc.compile()
    return nc


def _get_nc():
    global _nc_cache
    if _nc_cache is None:
        _nc_cache = _build_nc()
    return _nc_cache


def _numerator_host(em, tags, mask, trans, start, end):
    em64 = em.astype(np.float64)
    tags = tags.astype(np.int64)
    bidx = np.arange(em.shape[0])
    score = start.astype(np.float64)[tags[:, 0]] + em64[bidx, 0, tags[:, 0]]
    trans_term = trans.astype(np.float64)[tags[:, 1:], tags[:, :-1]]
    em_term = np.take_along_axis(em64[:, 1:], tags[:, 1:, None], axis=2)[..., 0]
    m = mask[:, 1:].astype(np.float64)
    score = score + ((trans_term + em_term) * m).sum(axis=1)
    last_idx = mask.sum(axis=1).astype(np.int64) - 1
    last_tags = np.take_along_axis(tags, last_idx[:, None], axis=1)[:, 0]
    return score + end.astype(np.float64)[last_tags]


def _reference_host(em, tags, mask, trans, start, end):
    """Pure-numpy fp64 fallback (exact semantics incl. arbitrary masks)."""
    em64 = em.astype(np.float64)
    score = start.astype(np.float64) + em64[:, 0]  # [B, T]
    t64 = trans.astype(np.float64)
    for i in range(1, em.shape[1]):
        x = score[:, :, None] + t64[None] + em64[:, i][:, None, :]
        mx = x.max(axis=1)
        nxt = mx + np.log(np.exp(x - mx[:, None, :]).sum(axis=1))
        score = np.where(mask[:, i][:, None], nxt, score)
    x = score + end.astype(np.float64)
    mx = x.max(axis=1, keepdims=True)
    denom = (mx[:, 0] + np.log(np.exp(x - mx).sum(axis=1)))
    numer = _numerator_host(em, tags, mask, trans, start, end)
    return np.float32((denom - numer).mean())


def kernel(**inputs):
    global LAST_RESULTS
    em = np.asarray(inputs["emissions"], dtype=np.float32)
    tags = np.asarray(inputs["tags"])
    mask = np.asarray(inputs["mask"])
    trans = np.asarray(inputs["transitions"], dtype=np.float32)
    start = np.asarray(inputs["start_transitions"], dtype=np.float32)
    end = np.asarray(inputs["end_transitions"], dtype=np.float32)

    if not mask.all():
        # device scan assumes a dense mask (guaranteed by the input spec);
        # fall back to the exact host path otherwise
        return _reference_host(em, tags, mask, trans, start, end)

    from concourse.bass_utils import run_bass_kernel_spmd

    nc = _get_nc()
    bf = ml_dtypes.bfloat16
    a_exp_np = np.exp(trans).astype(bf)
    se_np = np.stack([np.exp(start), np.exp(end)], axis=1).astype(np.float32)
    in_maps = []
    for cid in range(NCORES):
        emc = em[cid * BC:(cid + 1) * BC].astype(bf)       # [BC, S, T]
        em_t_np = np.ascontiguousarray(emc.transpose(2, 1, 0))  # [T, S, BC]
        in_maps.append({"em_t": em_t_np, "a_exp": a_exp_np, "se_exp": se_np})

    LAST_RESULTS = run_bass_kernel_spmd(nc, in_maps, list(range(NCORES)))
    denoms = np.concatenate(
        [LAST_RESULTS.results[cid]["denom"][0] for cid in range(NCORES)])

    numer = _numerator_host(em, tags, mask, trans, start, end)
    return np.float32((denoms.astype(np.float64) - numer).mean())


# revision 16
# speedup vs baseline: 1.3291x; 1.3291x over previous
"""CRF loss kernel for Trainium2 (8 NeuronCores, data-parallel over batch).

reference: mean_b( logZ_b - score_b ) for a linear-chain CRF with
B=256, S=512, T=128.

Denominator (logZ, 99.9% of the FLOPs) runs on device in exp space:
    u_0[t, b]   = exp(start[t]) * exp(em[b, 0, t])
    u_s         = (A^T u_{s-1}) o exp(em_s)          A = exp(transitions)
    logZ_b      = log( sum_t u_S[t,b] * exp(end[t]) ) + C
with a shared (per-core) renormalization scalar folded into the emission
tile every K=8 steps (prescaled off the critical path on the scalar
engine from a 3-step-delayed row sum), accumulated into C. Matmuls are
bf16 with fp32 PSUM accumulation; validated to ~2e-5 relative error
against the fp32 reference.

Layout per core: u is [T=128 partitions, batch free]. The per-step
matmul is out[next, b] = sum_prev A[prev, next] * u[prev, b] with A
stationary. The 32 per-core batches are split into G=2 groups of 16 so
the TensorE -> VectorE -> TensorE dependency chain of the two groups
interleaves; the steady-state step period is the chain-latency floor
(~430 ns: matmul drain + DVE PSUM-access + two semaphore hops).

Numerator (score of the tagged path) is a handful of gathers summing to
~0.1% of the FLOPs; it is computed on the host in fp64.
"""

import numpy as np
import ml_dtypes

B, S, T = 256, 512, 128
NCORES = 8
BC = B // NCORES          # 32 batches per core
G = 2                     # pipeline groups per core
BG = BC // G              # 16 batches per group
RENORM_K = 8              # renorm period (steps)
STAT_LAG = 3              # stat measured at s, applied at s + STAT_LAG
CH0 = 16                  # first chunk kept small so the scan starts early
CH = 64                   # emission chunk length (steps per DMA)

_nc_cache = None
LAST_RESULTS = None       # BassKernelResults of the most recent device run


def _chunks():
    """Chunk layout: (start, length) covering [0, S)."""
    out = [(0, CH0), (CH0, CH - CH0)]
    s = CH
    while s < S:
        out.append((s, CH))
        s += CH
    return out


def _build_nc():
    import concourse.bacc as bacc
    import concourse.mybir as mybir
    import concourse.tile as tile

    fp32 = mybir.dt.float32
    bf16 = mybir.dt.bfloat16
    Exp = mybir.ActivationFunctionType.Exp
    Ln = mybir.ActivationFunctionType.Ln
    mult = mybir.AluOpType.mult

    nc = bacc.Bacc("TRN2", target_bir_lowering=False, debug=False)

    em_t = nc.dram_tensor("em_t", [T, S, BC], bf16, kind="ExternalInput")
    a_exp = nc.dram_tensor("a_exp", [T, T], bf16, kind="ExternalInput")
    se_exp = nc.dram_tensor("se_exp", [T, 2], fp32, kind="ExternalInput")
    denom = nc.dram_tensor("denom", [1, BC], fp32, kind="ExternalOutput")

    chunks = _chunks()

    with tile.TileContext(nc) as tc:
        with (
            tc.tile_pool(name="const", bufs=1) as constp,
            tc.tile_pool(name="emraw", bufs=3) as emraw_p,
            # all exp(em) chunks stay resident (~4 MB of SBUF)
            tc.tile_pool(name="emexp", bufs=len(chunks)) as emexp_p,
            tc.tile_pool(name="emsc", bufs=3) as emsc_p,
            tc.tile_pool(name="ug0", bufs=2) as up0,
            tc.tile_pool(name="ug1", bufs=2) as up1,
            tc.tile_pool(name="vps", bufs=2, space="PSUM") as vp,
            tc.tile_pool(name="side", bufs=2) as sidep,
        ):
            ups = [up0, up1]

            emexp_tiles = []

            def load_chunk(ci):
                s0, ln = chunks[ci]
                raw = emraw_p.tile([T, ln, BC], bf16)
                nc.sync.dma_start(raw[:], em_t[:, s0:s0 + ln, :])
                ex = emexp_p.tile([T, ln, BC], bf16)
                nc.scalar.activation(ex[:], raw[:], Exp)
                emexp_tiles.append(ex)

            def em_slice(s):
                for ci, (s0, ln) in enumerate(chunks):
                    if s0 <= s < s0 + ln:
                        return emexp_tiles[ci][:, s - s0, :]
                raise AssertionError(s)

            load_chunk(0)

            a_tile = constp.tile([T, T], bf16)
            nc.sync.dma_start(a_tile[:], a_exp[:])
            se_tile = constp.tile([T, 2], fp32)
            nc.sync.dma_start(se_tile[:], se_exp[:])
            ones_t = constp.tile([T, 1], bf16)
            nc.gpsimd.memset(ones_t[:], 1.0)
            c_acc = sidep.tile([1, 1], fp32, tag="cacc")
            nc.gpsimd.memset(c_acc[:], 0.0)
            n_renorm = 0

            for ci in range(1, len(chunks)):
                load_chunk(ci)

            # u_0 = exp(em_0) * exp(start)
            u_cur = []
            for g in range(G):
                u0 = ups[g].tile([T, BG], bf16)
                nc.vector.tensor_scalar(
                    u0[:], em_slice(0)[:, g * BG:(g + 1) * BG],
                    se_tile[:, 0:1], None, mult)
                u_cur.append(u0)

            asum = None
            em_scaled = None
            for s in range(1, S):
                stat_step = ((s + STAT_LAG) % RENORM_K == 0
                             and s + STAT_LAG < S)
                renorm_step = (s % RENORM_K == 0)
                for g in range(G):
                    v = vp.tile([T, BG], fp32, tag=f"v{g}")
                    nc.tensor.matmul(v[:], a_tile[:], u_cur[g][:],
                                     start=True, stop=True)
                    if renorm_step:
                        em_ap = em_scaled[:, g * BG:(g + 1) * BG]
                    else:
                        em_ap = em_slice(s)[:, g * BG:(g + 1) * BG]
                    u_new = ups[g].tile([T, BG], bf16)
                    if stat_step and g == 0:
                        asum = sidep.tile([T, 1], fp32, tag="asum")
                        nc.vector.scalar_tensor_tensor(
                            u_new[:], v[:], 1.0, em_ap, mult, mult,
                            accum_out=asum[:])
                    else:
                        nc.vector.tensor_tensor(u_new[:], v[:], em_ap, mult)
                    u_cur[g] = u_new
                if stat_step:
                    # off-critical-path side chain. r = 1/asum[0] and the
                    # emission prescale run on DVE+GPSIMD only (the scan
                    # never waits on ACT); lg = ln(2^-32 * asum[0]) runs on
                    # ACT (Ln only — ACT function switches cost a ~1.3us
                    # table reload) and merely feeds the epilogue C. The
                    # 32*ln2 per renorm is a compile-time count folded into
                    # the epilogue constant.
                    r_row = sidep.tile([1, 1], fp32, tag="rrow")
                    nc.vector.reciprocal(r_row[:], asum[0:1, 0:1])
                    r_bc = sidep.tile([T, 1], fp32, tag="rbc")
                    nc.gpsimd.partition_broadcast(r_bc[:], r_row[:])
                    em_scaled = emsc_p.tile([T, BC], bf16)
                    nc.vector.tensor_scalar(
                        em_scaled[:], em_slice(s + STAT_LAG),
                        r_bc[:, 0:1], None, mult)
                    lg = sidep.tile([1, 1], fp32, tag="lg")
                    nc.scalar.activation(lg[:], asum[0:1, 0:1], Ln,
                                         scale=float(2.0 ** -32))
                    c_new = sidep.tile([1, 1], fp32, tag="cacc")
                    nc.vector.tensor_tensor(c_new[:], c_acc[:], lg[:],
                                            mybir.AluOpType.add)
                    c_acc = c_new
                    n_renorm += 1

            # epilogue: denom = ln(sum_t u_S * exp(end)) + C
            # (sum via a ones-vector matmul; all matmuls self-load weights,
            # so the ones load cannot corrupt any still-pending scan matmul)
            for g in range(G):
                w = ups[g].tile([T, BG], bf16, tag=f"w{g}")
                nc.vector.tensor_scalar(w[:], u_cur[g][:], se_tile[:, 1:2],
                                        None, mult)
                srow = vp.tile([1, BG], fp32, tag=f"sum{g}")
                nc.tensor.matmul(srow[:], ones_t[:], w[:], start=True, stop=True)
                dlog = sidep.tile([1, BG], fp32, tag=f"dlog{g}")
                nc.scalar.activation(dlog[:], srow[:], Ln,
                                     scale=float(2.0 ** -64))
                dfin = sidep.tile([1, BG], fp32, tag=f"dfin{g}")
                nc.vector.tensor_scalar(
                    dfin[:], dlog[:], c_acc[0:1, 0:1],
                    float((64 + 32 * n_renorm) * np.log(2.0)),
                    mybir.AluOpType.add, mybir.AluOpType.add)
                nc.sync.dma_start(denom[0:1, g * BG:(g + 1) * BG], dfin[:])

    nc.compile()
    return nc


def _get_nc():
    global _nc_cache
    if _nc_cache is None:
        _nc_cache = _build_nc()
    return _nc_cache


def _numerator_host(em, tags, mask, trans, start, end):
    em64 = em.astype(np.float64)
    tags = tags.astype(np.int64)
    bidx = np.arange(em.shape[0])
    score = start.astype(np.float64)[tags[:, 0]] + em64[bidx, 0, tags[:, 0]]
    trans_term = trans.astype(np.float64)[tags[:, 1:], tags[:, :-1]]
    em_term = np.take_along_axis(em64[:, 1:], tags[:, 1:, None], axis=2)[..., 0]
    m = mask[:, 1:].astype(np.float64)
    score = score + ((trans_term + em_term) * m).sum(axis=1)
    last_idx = mask.sum(axis=1).astype(np.int64) - 1
    last_tags = np.take_along_axis(tags, last_idx[:, None], axis=1)[:, 0]
    return score + end.astype(np.float64)[last_tags]


def _reference_host(em, tags, mask, trans, start, end):
    """Pure-numpy fp64 fallback (exact semantics incl. arbitrary masks)."""
    em64 = em.astype(np.float64)
    score = start.astype(np.float64) + em64[:, 0]  # [B, T]
    t64 = trans.astype(np.float64)
    for i in range(1, em.shape[1]):
        x = score[:, :, None] + t64[None] + em64[:, i][:, None, :]
        mx = x.max(axis=1)
        nxt = mx + np.log(np.exp(x - mx[:, None, :]).sum(axis=1))
        score = np.where(mask[:, i][:, None], nxt, score)
    x = score + end.astype(np.float64)
    mx = x.max(axis=1, keepdims=True)
    denom = (mx[:, 0] + np.log(np.exp(x - mx).sum(axis=1)))
    numer = _numerator_host(em, tags, mask, trans, start, end)
    return np.float32((denom - numer).mean())


def kernel(**inputs):
    global LAST_RESULTS
    em = np.asarray(inputs["emissions"], dtype=np.float32)
    tags = np.asarray(inputs["tags"])
    mask = np.asarray(inputs["mask"])
    trans = np.asarray(inputs["transitions"], dtype=np.float32)
    start = np.asarray(inputs["start_transitions"], dtype=np.float32)
    end = np.asarray(inputs["end_transitions"], dtype=np.float32)

    if not mask.all():
        # device scan assumes a dense mask (guaranteed by the input spec);
        # fall back to the exact host path otherwise
        return _reference_host(em, tags, mask, trans, start, end)

    from concourse.bass_utils import run_bass_kernel_spmd

    nc = _get_nc()
    bf = ml_dtypes.bfloat16
    a_exp_np = np.exp(trans).astype(bf)
    se_np = np.stack([np.exp(start), np.exp(end)], axis=1).astype(np.float32)
    in_maps = []
    for cid in range(NCORES):
        emc = em[cid * BC:(cid + 1) * BC].astype(bf)       # [BC, S, T]
        em_t_np = np.ascontiguousarray(emc.transpose(2, 1, 0))  # [T, S, BC]
        in_maps.append({"em_t": em_t_np, "a_exp": a_exp_np, "se_exp": se_np})

    LAST_RESULTS = run_bass_kernel_spmd(nc, in_maps, list(range(NCORES)))
    denoms = np.concatenate(
        [LAST_RESULTS.results[cid]["denom"][0] for cid in range(NCORES)])

    numer = _numerator_host(em, tags, mask, trans, start, end)
    return np.float32((denoms.astype(np.float64) - numer).mean())
